# revision 16
# baseline (speedup 1.0000x reference)
"""Trainium2 Bass kernel for nn_AttentionLayer (B=4, C=64, N=4096, dk=64).

Math (per batch b):
    q_t[d, n] = (Wq/8) @ x[b]          # [64, N]
    k[d, m]   = Wk @ x[b]              # [64, N]
    v_t[n, o] = (Wv @ x[b]).T          # [N, 64]
    s[n, m]   = q_t.T @ k              # [N, N]
    attn      = softmax over n (columns)
    out[o, m] = v.T @ attn             # [64, N]

Sharding: 8 cores = 4 batches x 2 column-halves; core (b, h) computes
out[b, :, h*2048:(h+1)*2048]. The softmax axis n is fully local -> no
collectives. The tiny projections (0.25% of FLOPs) run on host so the
device inputs can be fed pre-laid-out in the matmul dtypes.

Device kernel per core (the N^2 part):
  - scores: TensorE fp16 matmuls [128x512] into [128, grp*512] PSUM
    groups (grp banks, double-buffered so TensorE isn't WAR-blocked)
  - exp: ScalarE straight out of PSUM, exp(s - ln4) via the free ACT
    bias (keeps e' <= ~66, under the TRN fp8e4 +-240 ceiling), written
    directly as fp8e4 into exp_sb [128, NCH, 512]
  - AV: fp8 DoubleRow pair-matmuls: lhsT = v pairs [128, 2, 65] (chunk
    stride padded to 80 B for the LDW step%16 rule, 65th col = ones ->
    colsum), rhs = exp pairs [128, 2, 512], accumulated into PSUM
    [65, 512] f32 over 16 pairs. DoubleRow feeds 2 fp8 contraction rows
    per cycle -> ~1.8x the fp16 AV rate.
  - AV pair-matmuls of m-tile t-1 are interleaved between the score
    groups of m-tile t so the PE never idles (HAM stays at K=8/8).
  - out DMA [65, 512] per m-tile: rows 0:64 = numerator, row 64 = colsum.
Host divides numerator by colsum and reassembles the full output.

PE work per core: scores 65536 cyc + AV ~37k cyc (vs 65536 fp16) at
2.4 GHz. rel_err 7.221e-3 vs the f64 reference (fp8 quantization of v
and exp; gate is 2e-2). The all-fp16 path (prec="fp16", ~131072 cyc) is
kept for A/B.

Measured (loop-slope over For_i reps 2048/18432, median): 94.6 us/iter
for BEST (grp=43, exp_bufs=3) vs 108.5 us/iter for the previous all-fp16
kernel under the same method. At these rep counts the PE sits in its
sustained-load ~1.2 GHz clamp, so both figures are conservative; short
single executions run the PE at 2.4 GHz where the cycle accounting gives
~43 us PE / ~65 us ScalarE-exp (1 elem/cycle/lane at 1.2 GHz, measured —
the exp, not the matmuls, bounds a single warm execution).
"""

import ml_dtypes
import numpy as np

import concourse.bass as bass  # noqa: F401  (registers engine methods)
import concourse.mybir as mybir
import concourse.tile as tile
from concourse import bacc
from concourse.bass_utils import run_bass_kernel_spmd

B, C, N = 4, 64, 4096
MLOC = N // 2            # columns per core
P = 128
NCH = N // P             # 32 row-chunks of the score matrix
MT = 512                 # m-tile width (PSUM free dim)
NMT = MLOC // MT         # 4 m-tiles per core
GRP = 3                  # score chunks exp'd per ScalarE instruction
CP1 = C + 1              # v columns + ones column
VP = 80                  # padded v chunk stride (bytes, %16==0) for DoubleRow LDW
EXP_BIAS = -1.3862943611198906  # -ln(4): cancels in num/den, keeps e' in fp8 range

F32 = mybir.dt.float32
BF16 = mybir.dt.bfloat16
FP16 = mybir.dt.float16
F8 = mybir.dt.float8e4
EXP = mybir.ActivationFunctionType.Exp
DROW = mybir.MatmulPerfMode.DoubleRow

_NC_CACHE = {}


def _build(grp=GRP, spsum_bufs=2, exp_bufs=2, prec="fp8av", ilv=True,
           avd=1.0, loop_reps=None):
    """Build the per-core graph.

    grp: score chunks per exp instruction ([128, grp*512] PSUM group).
    spsum_bufs: score-PSUM group buffers (grp*spsum_bufs + 2 <= 8 banks).
    exp_bufs: exp_sb SBUF buffers.
    prec: "fp16" (all fp16, PE ~131072 cyc/iter) or "fp8av" (fp16 scores,
        fp8e4 exp/v with DoubleRow AV, PE ~103k cyc/iter).
    ilv: interleave AV matmuls of m-tile t-1 between score groups of
        m-tile t (keeps the PE busy while ACT catches up on exp).
    loop_reps: if set, wrap the attention body in a hardware For_i loop
        (used only for timing: per-iteration time = slope over reps).
    """
    fp8 = prec == "fp8av"
    qk_dt = FP16
    ev_dt = F8 if fp8 else {"fp16": FP16, "bf16": BF16}[prec]
    vp = VP if fp8 else CP1
    nc = bacc.Bacc("TRN2", target_bir_lowering=False, debug=False)
    q_ext = nc.declare_dram_parameter("q", [C, N], qk_dt, isOutput=False)
    k_ext = nc.declare_dram_parameter("k", [C, MLOC], qk_dt, isOutput=False)
    v_ext = nc.declare_dram_parameter("v", [P, NCH, vp], ev_dt, isOutput=False)
    out_ext = nc.declare_dram_parameter("out", [CP1, MLOC], F32, isOutput=True)

    # n-chunk groups per m-tile.
    if grp == 43:
        # alternating {4,3} groups in a 7-bank ring (4+3+1 AV = 8 banks,
        # each group set single-buffered but the two alternate -> the PE
        # writes one while ACT reads the other). 9 ACT instrs per m-tile.
        gsizes = [4, 3, 4, 3, 4, 3, 4, 3, 4]
        apool_bufs = 1
    else:
        assert grp * spsum_bufs + 2 <= 8
        gsizes = []
        left = NCH
        while left > 0:
            gsizes.append(min(grp, left))
            left -= gsizes[-1]
        apool_bufs = 2
    assert sum(gsizes) == NCH

    with tile.TileContext(nc) as tc:
        with (
            tc.tile_pool(name="const", bufs=1) as cpool,
            tc.tile_pool(name="expp", bufs=exp_bufs) as epool,
            tc.tile_pool(name="outp", bufs=2) as opool,
            tc.tile_pool(name="spsumA", bufs=1 if grp == 43 else spsum_bufs,
                         space="PSUM") as spoolA,
            tc.tile_pool(name="spsumB", bufs=1, space="PSUM") as spoolB,
            tc.tile_pool(name="apsum", bufs=apool_bufs, space="PSUM") as apool,
        ):
            # One serial HWDGE queue -> emit in first-needed order: the first
            # scores group needs q[:, :384] and k[:, :512]; v is needed a few
            # us in (first AV matmul); later k/q chunks are consumed later.
            k_sb = cpool.tile([C, MLOC], qk_dt)
            q_sb = cpool.tile([C, N], qk_dt)
            v_sb = cpool.tile([P, NCH, vp], ev_dt)
            bias_sb = cpool.tile([P, 1], F32)
            nc.gpsimd.memset(bias_sb[:], EXP_BIAS)

            def dq(j):
                nc.sync.dma_start(
                    q_sb[:, j * 512:(j + 1) * 512], q_ext[:, j * 512:(j + 1) * 512]
                )

            def dk(j):
                nc.sync.dma_start(
                    k_sb[:, j * 512:(j + 1) * 512], k_ext[:, j * 512:(j + 1) * 512]
                )

            def dv(j):
                nc.sync.dma_start(
                    v_sb[:, j * 8:(j + 1) * 8, :], v_ext[:, j * 8:(j + 1) * 8, :]
                )

            dq(0); dk(0); dq(1); dv(0); dq(2); dv(1); dq(3); dv(2)
            dq(4); dv(3); dq(5); dq(6); dq(7); dk(1); dk(2); dk(3)

            # Single-shot warmup (outside any timing loop): ~10 junk
            # matmuls over the just-DMA'd q chunk keep TensorE busy during
            # the input stream so the HAM clock-gate reaches K=8/8 before
            # real work, and one tiny exp right after the first matmul
            # pulls the ~2.7us ACT table load into the DMA shadow.
            wps = spoolA.tile([P, 4 if grp == 43 else grp, MT], F32, tag="sc")
            wsc = cpool.tile([P, 1], F8 if fp8 else FP16)
            for w in range(10):
                nc.tensor.matmul(
                    wps[:, 0, :], lhsT=q_sb[:, :P], rhs=q_sb[:, :MT],
                    start=True, stop=True,
                )
                if w == 0:
                    nc.scalar.activation(wsc[:], wps[:, 0, :1], EXP,
                                         bias=bias_sb[:] if fp8 else 0.0)

            def q_ap(i):
                return q_sb[:, i * P:(i + 1) * P]

            def k_ap(t):
                return k_sb[:, t * MT:(t + 1) * MT]

            def av_units(t, exp_sb, pav):
                """AV matmul emitters for m-tile t (accumulate into pav)."""
                if fp8:
                    npair = NCH // 2

                    def mk(i):
                        def emit():
                            nc.tensor.matmul(
                                pav[:],
                                lhsT=v_sb[:, 2 * i:2 * i + 2, :CP1],
                                rhs=exp_sb[:, 2 * i:2 * i + 2, :],
                                start=(i == 0),
                                stop=(i == npair - 1),
                                perf_mode=DROW,
                            )
                        return emit

                    return [mk(i) for i in range(npair)]

                def mk(i):
                    def emit():
                        nc.tensor.matmul(
                            pav[:],
                            lhsT=v_sb[:, i, :CP1],
                            rhs=exp_sb[:, i, :],
                            start=(i == 0),
                            stop=(i == NCH - 1),
                        )
                    return emit

                return [mk(i) for i in range(NCH)]

            def finish_mtile(t, pav):
                o_sb = opool.tile([CP1, MT], F32, tag="ot")
                nc.vector.tensor_copy(o_sb[:], pav[:])
                nc.sync.dma_start(out_ext[:, t * MT:(t + 1) * MT], o_sb[:])

            def attention_body(iv=None):
                prev = None  # (t-1, its pending AV units, its pav)
                for t in range(NMT):
                    exp_sb = epool.tile([P, NCH, MT], ev_dt, tag="exp")
                    i = 0
                    ng = len(gsizes)
                    for g, gs in enumerate(gsizes):
                        if grp == 43:
                            pool = spoolA if g % 2 == 0 else spoolB
                            ps = pool.tile([P, gs, MT], F32, tag="sc")
                        else:
                            ps = spoolA.tile([P, grp, MT], F32, tag="sc")
                        for u in range(gs):
                            nc.tensor.matmul(
                                ps[:, u, :],
                                lhsT=q_ap(i + u),
                                rhs=k_ap(t),
                                start=True,
                                stop=True,
                            )
                        nc.scalar.activation(
                            exp_sb[:, i:i + gs, :], ps[:, :gs, :], EXP,
                            bias=bias_sb[:] if fp8 else 0.0,
                        )
                        i += gs
                        if ilv and prev is not None:
                            pt, units, nu, ppav = prev
                            # consume the deferred AV units across the first
                            # avd-fraction of this m-tile's score groups
                            nga = max(1, int(round(ng * avd)))
                            ge = min(g + 1, nga)
                            take = ge * nu // nga - g * nu // nga if g < nga \
                                else 0
                            for _ in range(take):
                                units.pop(0)()
                            if g == ng - 1:
                                assert not units, (g, ng, nga, len(units))
                                finish_mtile(pt, ppav)
                    pav = apool.tile([CP1, MT], F32, tag="av")
                    units = av_units(t, exp_sb, pav)
                    if ilv:
                        prev = (t, units, len(units), pav)
                    else:
                        for emit in units:
                            emit()
                        finish_mtile(t, pav)
                if ilv and prev is not None:
                    pt, units, nu, ppav = prev
                    for emit in units:
                        emit()
                    finish_mtile(pt, ppav)

            if loop_reps is None:
                attention_body()
            else:
                # PE body is ~384 instructions (> one IRAM block): arm the
                # back-edge branch hint so each iteration I$-hits.
                with tc.For_i(0, loop_reps, 1,
                              hint_engines=(mybir.EngineType.PE,)):
                    attention_body()

    nc.compile()
    return nc


BEST = {"grp": 43, "exp_bufs": 3, "prec": "fp8av", "ilv": True}


def _get_nc():
    if "nc" not in _NC_CACHE:
        _NC_CACHE["nc"] = _build(**BEST)
    return _NC_CACHE["nc"]


def _make_in_maps(x, Wq, Wk, Wv, prec="fp8av"):
    fp8 = prec == "fp8av"
    ev_np = ml_dtypes.float8_e4m3fn if fp8 else (
        np.float16 if prec == "fp16" else ml_dtypes.bfloat16)
    vp = VP if fp8 else CP1
    x = np.asarray(x, np.float32)
    wq8 = np.asarray(Wq, np.float32) * 0.125
    wk = np.asarray(Wk, np.float32)
    wv = np.asarray(Wv, np.float32)
    in_maps = []
    for b in range(B):
        xb = x[b]                                  # [C, N]
        qt = np.ascontiguousarray(wq8 @ xb)        # [C, N]
        kf = wk @ xb                               # [C, N]
        vt = (wv @ xb).T                           # [N, C]
        va = np.zeros((P, NCH, vp), np.float32)
        v3 = vt.reshape(NCH, P, C).transpose(1, 0, 2)   # [P, NCH, C]
        va[:, :, :C] = v3
        va[:, :, C] = 1.0
        va = va.astype(ev_np)
        for h in range(2):
            in_maps.append(
                {
                    "q": qt.astype(np.float16),
                    "k": np.ascontiguousarray(
                        kf[:, h * MLOC:(h + 1) * MLOC]
                    ).astype(np.float16),
                    "v": va,
                }
            )
    return in_maps


def _assemble(results):
    out = np.empty((B, C, N), np.float32)
    for core in range(2 * B):
        b, h = divmod(core, 2)
        r = results[core]["out"]
        out[b, :, h * MLOC:(h + 1) * MLOC] = r[:C] / r[C:C + 1]
    return out


def run(x, Wq, Wk, Wv, trace=False, **trace_kwargs):
    nc = _get_nc()
    res = run_bass_kernel_spmd(
        nc,
        _make_in_maps(x, Wq, Wk, Wv, prec=BEST.get("prec", "fp8av")),
        core_ids=list(range(2 * B)),
        trace=trace,
        **trace_kwargs,
    )
    return _assemble(res.results), res


def kernel(x, Wq, Wk, Wv):
    out, _ = run(x, Wq, Wk, Wv, trace=False)
    return out


# revision 23
# speedup vs baseline: 1.2896x; 1.2896x over previous
"""Trainium2 Bass kernel for nn_AttentionLayer (B=4, C=64, N=4096, dk=64).

Math (per batch b):
    q_t[d, n] = (Wq/8) @ x[b]          # [64, N]
    k[d, m]   = Wk @ x[b]              # [64, N]
    v_t[n, o] = (Wv @ x[b]).T          # [N, 64]
    s[n, m]   = q_t.T @ k              # [N, N]
    attn      = softmax over n (columns)
    out[o, m] = v.T @ attn             # [64, N]

Sharding: 8 cores = 4 batches x 2 column-halves; core (b, h) computes
out[b, :, h*2048:(h+1)*2048]. The softmax axis n is fully local -> no
collectives. The tiny projections (0.25% of FLOPs) run on host so the
device inputs can be fed pre-laid-out in the matmul dtypes.

Device kernel per core (the N^2 part):
  - scores: TensorE fp16 matmuls [128x512] into [128, grp*512] PSUM
    groups (grp banks, double-buffered so TensorE isn't WAR-blocked)
  - exp: ScalarE straight out of PSUM, exp(s - ln4) via the free ACT
    bias (keeps e' <= ~66, under the TRN fp8e4 +-240 ceiling), written
    directly as fp8e4 into exp_sb [128, NCH, 512]
  - AV: fp8 DoubleRow pair-matmuls: lhsT = v pairs [128, 2, 65] (chunk
    stride padded to 80 B for the LDW step%16 rule, 65th col = ones ->
    colsum), rhs = exp pairs [128, 2, 512], accumulated into PSUM
    [65, 512] f32 over 16 pairs. DoubleRow feeds 2 fp8 contraction rows
    per cycle -> ~1.8x the fp16 AV rate.
  - AV pair-matmuls of m-tile t-1 are interleaved between the score
    groups of m-tile t so the PE never idles (HAM stays at K=8/8).
  - out DMA [65, 512] per m-tile: rows 0:64 = numerator, row 64 = colsum.
Host divides numerator by colsum and reassembles the full output.

PE work per core: scores 65536 cyc + AV ~37k cyc (vs 65536 fp16) at
2.4 GHz. rel_err 7.221e-3 vs the f64 reference (fp8 quantization of v
and exp; gate is 2e-2). The all-fp16 path (prec="fp16", ~131072 cyc) is
kept for A/B.

prec="fp8qk" (BEST) moves the scores matmul onto the fp8 DoubleRow path
too, at full precision: q and k ship as fp8 (value, residual) pairs —
partitions 0:64 hold q8/k8, 64:128 hold the fp8-quantized residuals, and
the DoubleRow middle dim duplicates them — so one pair-matmul computes
all four terms of (q8+dq8)^T (k8+dk8), i.e. exact scores up to the
~0.1%-of-q residual rounding. The /8 softmax scaling rides on the ACT
scale operand so the residuals stay clear of the fp8 subnormal floor.
Measured pure-PE stream rates: fp16 N=512 matmul 446 ns vs fp8-DoubleRow
254 ns (the DR stream sustains the full 2.4 GHz rate; the fp16 stream
does not), which is why this wins ~7 us despite equal element counts.

Measured (loop-slope over For_i reps 2048/18432, median): 86.3 us/iter
for BEST (grp=43, exp_bufs=3, fp8qk) vs 94.6 us/iter for fp8av (fp16
scores) and 108.5 us/iter for the original all-fp16 kernel under the
same method. rel_err 7.264e-3 (sim-identical). At these rep counts the
PE sits in its sustained-load regime, so the figures are conservative;
a short single execution is ScalarE-exp bound (~65 us: 1 elem/cycle/lane
at 1.2 GHz, measured) with all PE work (~32 us) hidden under it.
"""

import ml_dtypes
import numpy as np

import concourse.bass as bass  # noqa: F401  (registers engine methods)
import concourse.mybir as mybir
import concourse.tile as tile
from concourse import bacc
from concourse.bass_utils import run_bass_kernel_spmd

B, C, N = 4, 64, 4096
MLOC = N // 2            # columns per core
P = 128
NCH = N // P             # 32 row-chunks of the score matrix
MT = 512                 # m-tile width (PSUM free dim)
NMT = MLOC // MT         # 4 m-tiles per core
GRP = 3                  # score chunks exp'd per ScalarE instruction
CP1 = C + 1              # v columns + ones column
VP = 80                  # padded v chunk stride (bytes, %16==0) for DoubleRow LDW
EXP_BIAS = -1.3862943611198906  # -ln(4): cancels in num/den, keeps e' in fp8 range

F32 = mybir.dt.float32
BF16 = mybir.dt.bfloat16
FP16 = mybir.dt.float16
F8 = mybir.dt.float8e4
EXP = mybir.ActivationFunctionType.Exp
DROW = mybir.MatmulPerfMode.DoubleRow

_NC_CACHE = {}


def _build(grp=GRP, spsum_bufs=2, exp_bufs=2, prec="fp8av", ilv=True,
           avd=1.0, loop_reps=None):
    """Build the per-core graph.

    grp: score chunks per exp instruction ([128, grp*512] PSUM group).
    spsum_bufs: score-PSUM group buffers (grp*spsum_bufs + 2 <= 8 banks).
    exp_bufs: exp_sb SBUF buffers.
    prec: "fp16" (all fp16, PE ~131072 cyc/iter) or "fp8av" (fp16 scores,
        fp8e4 exp/v with DoubleRow AV, PE ~103k cyc/iter).
    ilv: interleave AV matmuls of m-tile t-1 between score groups of
        m-tile t (keeps the PE busy while ACT catches up on exp).
    loop_reps: if set, wrap the attention body in a hardware For_i loop
        (used only for timing: per-iteration time = slope over reps).
    """
    fp8 = prec in ("fp8av", "fp8qk")
    fp8qk = prec == "fp8qk"
    qk_dt = FP16
    ev_dt = F8 if fp8 else {"fp16": FP16, "bf16": BF16}[prec]
    vp = VP if fp8 else CP1
    nc = bacc.Bacc("TRN2", target_bir_lowering=False, debug=False)
    if fp8qk:
        # q/k as fp8 (value, residual) pairs in DoubleRow layout: one
        # pair-matmul computes all four terms of (q8+dq8)^T (k8+dk8) —
        # full-precision scores on the 2x-rate fp8 path. Partitions 0:64
        # carry the value, 64:128 the residual; the middle dim duplicates
        # data so the same AP serves both DoubleRow halves.
        q_ext = nc.declare_dram_parameter("q", [P, NCH, 2, P], F8,
                                          isOutput=False)
        k_ext = nc.declare_dram_parameter("k", [P, 2, MLOC], F8,
                                          isOutput=False)
    else:
        q_ext = nc.declare_dram_parameter("q", [C, N], qk_dt, isOutput=False)
        k_ext = nc.declare_dram_parameter("k", [C, MLOC], qk_dt,
                                          isOutput=False)
    v_ext = nc.declare_dram_parameter("v", [P, NCH, vp], ev_dt, isOutput=False)
    out_ext = nc.declare_dram_parameter("out", [CP1, MLOC], F32, isOutput=True)

    # n-chunk groups per m-tile.
    if grp == 43:
        # alternating {4,3} groups in a 7-bank ring (4+3+1 AV = 8 banks,
        # each group set single-buffered but the two alternate -> the PE
        # writes one while ACT reads the other). 9 ACT instrs per m-tile.
        gsizes = [4, 3, 4, 3, 4, 3, 4, 3, 4]
        apool_bufs = 1
    else:
        assert grp * spsum_bufs + 2 <= 8
        gsizes = []
        left = NCH
        while left > 0:
            gsizes.append(min(grp, left))
            left -= gsizes[-1]
        apool_bufs = 2
    assert sum(gsizes) == NCH

    with tile.TileContext(nc) as tc:
        with (
            tc.tile_pool(name="const", bufs=1) as cpool,
            tc.tile_pool(name="expp", bufs=exp_bufs) as epool,
            tc.tile_pool(name="outp", bufs=2) as opool,
            tc.tile_pool(name="spsumA", bufs=1 if grp == 43 else spsum_bufs,
                         space="PSUM") as spoolA,
            tc.tile_pool(name="spsumB", bufs=1, space="PSUM") as spoolB,
            tc.tile_pool(name="apsum", bufs=apool_bufs, space="PSUM") as apool,
        ):
            # One serial HWDGE queue -> emit in first-needed order: the first
            # scores group needs q[:, :384] and k[:, :512]; v is needed a few
            # us in (first AV matmul); later k/q chunks are consumed later.
            if fp8qk:
                q_sb = cpool.tile([P, NCH, 2, P], F8)
                k_sb = cpool.tile([P, 2, MLOC], F8)
            else:
                q_sb = cpool.tile([C, N], qk_dt)
                k_sb = cpool.tile([C, MLOC], qk_dt)
            v_sb = cpool.tile([P, NCH, vp], ev_dt)
            bias_sb = cpool.tile([P, 1], F32)
            nc.gpsimd.memset(bias_sb[:], EXP_BIAS)

            def dq(j):
                if fp8qk:
                    nc.sync.dma_start(q_sb[:, j * 4:(j + 1) * 4, :, :],
                                      q_ext[:, j * 4:(j + 1) * 4, :, :])
                else:
                    nc.sync.dma_start(q_sb[:, j * 512:(j + 1) * 512],
                                      q_ext[:, j * 512:(j + 1) * 512])

            def dk(j):
                if fp8qk:
                    nc.sync.dma_start(k_sb[:, :, j * 512:(j + 1) * 512],
                                      k_ext[:, :, j * 512:(j + 1) * 512])
                else:
                    nc.sync.dma_start(k_sb[:, j * 512:(j + 1) * 512],
                                      k_ext[:, j * 512:(j + 1) * 512])

            def dv(j):
                nc.sync.dma_start(
                    v_sb[:, j * 8:(j + 1) * 8, :], v_ext[:, j * 8:(j + 1) * 8, :]
                )

            dq(0); dk(0); dq(1); dv(0); dq(2); dv(1); dq(3); dv(2)
            dq(4); dv(3); dq(5); dq(6); dq(7); dk(1); dk(2); dk(3)

            # Single-shot warmup (outside any timing loop): ~10 junk
            # matmuls over the just-DMA'd q chunk keep TensorE busy during
            # the input stream so the HAM clock-gate reaches K=8/8 before
            # real work, and one tiny exp right after the first matmul
            # pulls the ~2.7us ACT table load into the DMA shadow.
            wps = spoolA.tile([P, 4 if grp == 43 else grp, MT], F32, tag="sc")
            wsc = cpool.tile([P, 1], F8 if fp8 else FP16)
            for w in range(10):
                if fp8qk:
                    nc.tensor.matmul(
                        wps[:, 0, :], lhsT=q_sb[:, 0, :, :],
                        rhs=k_sb[:, :, :MT], start=True, stop=True,
                        perf_mode=DROW,
                    )
                else:
                    nc.tensor.matmul(
                        wps[:, 0, :], lhsT=q_sb[:, :P], rhs=q_sb[:, :MT],
                        start=True, stop=True,
                    )
                if w == 0:
                    nc.scalar.activation(wsc[:], wps[:, 0, :1], EXP,
                                         bias=bias_sb[:] if fp8 else 0.0)

            def q_ap(i):
                if fp8qk:
                    return q_sb[:, i, :, :]
                return q_sb[:, i * P:(i + 1) * P]

            def k_ap(t):
                if fp8qk:
                    return k_sb[:, :, t * MT:(t + 1) * MT]
                return k_sb[:, t * MT:(t + 1) * MT]

            def av_units(t, exp_sb, pav):
                """AV matmul emitters for m-tile t (accumulate into pav)."""
                if fp8:
                    npair = NCH // 2

                    def mk(i):
                        def emit():
                            nc.tensor.matmul(
                                pav[:],
                                lhsT=v_sb[:, 2 * i:2 * i + 2, :CP1],
                                rhs=exp_sb[:, 2 * i:2 * i + 2, :],
                                start=(i == 0),
                                stop=(i == npair - 1),
                                perf_mode=DROW,
                            )
                        return emit

                    return [mk(i) for i in range(npair)]

                def mk(i):
                    def emit():
                        nc.tensor.matmul(
                            pav[:],
                            lhsT=v_sb[:, i, :CP1],
                            rhs=exp_sb[:, i, :],
                            start=(i == 0),
                            stop=(i == NCH - 1),
                        )
                    return emit

                return [mk(i) for i in range(NCH)]

            def finish_mtile(t, pav):
                o_sb = opool.tile([CP1, MT], F32, tag="ot")
                nc.vector.tensor_copy(o_sb[:], pav[:])
                nc.sync.dma_start(out_ext[:, t * MT:(t + 1) * MT], o_sb[:])

            def attention_body(iv=None):
                prev = None  # (t-1, its pending AV units, its pav)
                for t in range(NMT):
                    exp_sb = epool.tile([P, NCH, MT], ev_dt, tag="exp")
                    i = 0
                    ng = len(gsizes)
                    for g, gs in enumerate(gsizes):
                        if grp == 43:
                            pool = spoolA if g % 2 == 0 else spoolB
                            ps = pool.tile([P, gs, MT], F32, tag="sc")
                        else:
                            ps = spoolA.tile([P, grp, MT], F32, tag="sc")
                        for u in range(gs):
                            nc.tensor.matmul(
                                ps[:, u, :],
                                lhsT=q_ap(i + u),
                                rhs=k_ap(t),
                                start=True,
                                stop=True,
                                perf_mode=DROW if fp8qk else None,
                            )
                        nc.scalar.activation(
                            exp_sb[:, i:i + gs, :], ps[:, :gs, :], EXP,
                            bias=bias_sb[:] if fp8 else 0.0,
                            scale=0.125 if fp8qk else 1.0,
                        )
                        i += gs
                        if ilv and prev is not None:
                            pt, units, nu, ppav = prev
                            # consume the deferred AV units across the first
                            # avd-fraction of this m-tile's score groups
                            nga = max(1, int(round(ng * avd)))
                            ge = min(g + 1, nga)
                            take = ge * nu // nga - g * nu // nga if g < nga \
                                else 0
                            for _ in range(take):
                                units.pop(0)()
                            if g == ng - 1:
                                assert not units, (g, ng, nga, len(units))
                                finish_mtile(pt, ppav)
                    pav = apool.tile([CP1, MT], F32, tag="av")
                    units = av_units(t, exp_sb, pav)
                    if ilv:
                        prev = (t, units, len(units), pav)
                    else:
                        for emit in units:
                            emit()
                        finish_mtile(t, pav)
                if ilv and prev is not None:
                    pt, units, nu, ppav = prev
                    for emit in units:
                        emit()
                    finish_mtile(pt, ppav)

            if loop_reps is None:
                attention_body()
            else:
                # PE body is ~384 instructions (> one IRAM block): arm the
                # back-edge branch hint so each iteration I$-hits.
                with tc.For_i(0, loop_reps, 1,
                              hint_engines=(mybir.EngineType.PE,)):
                    attention_body()

    nc.compile()
    return nc


BEST = {"grp": 43, "exp_bufs": 3, "prec": "fp8qk", "ilv": True}


def _get_nc():
    if "nc" not in _NC_CACHE:
        _NC_CACHE["nc"] = _build(**BEST)
    return _NC_CACHE["nc"]


def _make_in_maps(x, Wq, Wk, Wv, prec="fp8av"):
    fp8 = prec in ("fp8av", "fp8qk")
    fp8qk = prec == "fp8qk"
    E4 = ml_dtypes.float8_e4m3fn
    ev_np = E4 if fp8 else (
        np.float16 if prec == "fp16" else ml_dtypes.bfloat16)
    vp = VP if fp8 else CP1
    x = np.asarray(x, np.float32)
    # fp8qk ships q/k unscaled (the /8 rides on the ACT scale operand so
    # the fp8 residuals stay clear of the subnormal floor).
    wq8 = np.asarray(Wq, np.float32) * (1.0 if fp8qk else 0.125)
    wk = np.asarray(Wk, np.float32)
    wv = np.asarray(Wv, np.float32)

    def split8(a):
        hi = a.astype(E4)
        lo = (a - hi.astype(np.float32)).astype(E4)
        return hi, lo

    in_maps = []
    for b in range(B):
        xb = x[b]                                  # [C, N]
        qt = np.ascontiguousarray(wq8 @ xb)        # [C, N]
        kf = wk @ xb                               # [C, N]
        vt = (wv @ xb).T                           # [N, C]
        va = np.zeros((P, NCH, vp), np.float32)
        v3 = vt.reshape(NCH, P, C).transpose(1, 0, 2)   # [P, NCH, C]
        va[:, :, :C] = v3
        va[:, :, C] = 1.0
        va = va.astype(ev_np)
        if fp8qk:
            q8, dq8 = split8(qt)                   # [C, N] each
            qs = np.empty((P, NCH, 2, P), E4)
            qs[:C, :, 0, :] = qs[:C, :, 1, :] = \
                q8.reshape(C, NCH, P)
            qs[C:, :, 0, :] = qs[C:, :, 1, :] = \
                dq8.reshape(C, NCH, P)
        for h in range(2):
            kh = np.ascontiguousarray(kf[:, h * MLOC:(h + 1) * MLOC])
            if fp8qk:
                k8, dk8 = split8(kh)
                ks = np.empty((P, 2, MLOC), E4)
                ks[:C, 0, :] = ks[C:, 0, :] = k8
                ks[:C, 1, :] = ks[C:, 1, :] = dk8
                in_maps.append({"q": qs, "k": ks, "v": va})
            else:
                in_maps.append(
                    {
                        "q": qt.astype(np.float16),
                        "k": kh.astype(np.float16),
                        "v": va,
                    }
                )
    return in_maps


def _assemble(results):
    out = np.empty((B, C, N), np.float32)
    for core in range(2 * B):
        b, h = divmod(core, 2)
        r = results[core]["out"]
        out[b, :, h * MLOC:(h + 1) * MLOC] = r[:C] / r[C:C + 1]
    return out


def run(x, Wq, Wk, Wv, trace=False, **trace_kwargs):
    nc = _get_nc()
    res = run_bass_kernel_spmd(
        nc,
        _make_in_maps(x, Wq, Wk, Wv, prec=BEST.get("prec", "fp8av")),
        core_ids=list(range(2 * B)),
        trace=trace,
        **trace_kwargs,
    )
    return _assemble(res.results), res


def kernel(x, Wq, Wk, Wv):
    out, _ = run(x, Wq, Wk, Wv, trace=False)
    return out
